# revision 28
# baseline (speedup 1.0000x reference)
"""DLRM inference kernel for 8 Trainium2 NeuronCores (v2, fp16 data path).

Strategy: pure data parallelism. The batch (16384) is split into 8 slices
of 2048; embedding tables and MLP weights are replicated, so no
collectives. Eval-mode BatchNorm is folded into weights on the host, and
all tensor data is fp16 on device (fp32 PSUM accumulation): fp16 matmuls
run 4x faster than fp32 on the PE, and fp16 tensor_tensor ops get the
DVE's 2-byte 2x mode.

Per 128-sample tile:
  - Embedding rows arrive via per-column indirect DMAs (one [128,1]
    offset column per feature; wider offset APs are unreliable on the
    SWDGE ucode). The gathers are the only work on the Pool engine so
    its queue streams them continuously, prefetched two chunks ahead;
    the bottom-MLP output is DMA-transposed into slot 0.
  - Pairwise interactions use the shifted-delta trick entirely on the
    DVE with fp16 tensor_tensor ops (2-byte 2x mode): per-delta multiply
    then contiguous-halves adds (64->32->16), small deltas batched
    through a shared full-width buffer, then a global 4-level add tree
    (16->1). TensorReduce is avoided (no 2-byte speedup).
  - Sample-major -> feature-major transposes run on the PE (fp16,
    1 cyc/row) with Activation-engine copies out of PSUM.
"""

import sys

for _p in ("/opt/trn_rl_repo",):
    if _p not in sys.path:
        sys.path.insert(0, _p)

import numpy as np

import bass_rust
import concourse.bass as bass
import concourse.mybir as mybir
import concourse.tile as tile

# Problem constants (hardcoded per spec nn_DLRM_5403068858958)
B, CD, NF, V, D = 16384, 13, 26, 100000, 64
NCORES = 8
BC = B // NCORES          # 2048 samples per core
BN_INV = 1.0 / np.sqrt(1.0 + 1e-5)
P = 128
NE = NF + 1               # 27 concatenated features (bottom + 26 embeddings)
NEP = NE + 1              # padded to 28 features -> 1792 flat rows
FLAT_ROWS = NEP * D       # 1792 (14 k-tiles)
NPAIR = NE * (NE - 1) // 2  # 351
NPP = NPAIR + 1           # 352 (pair rows + 1 pad)
INTER_ROWS = 384          # 351 padded to 3 k-tiles
K_TILES = (FLAT_ROWS + INTER_ROWS) // P  # 17
TOPK = FLAT_ROWS + INTER_ROWS            # 2176
F32 = mybir.dt.float32
F16 = mybir.dt.float16
I32 = mybir.dt.int32
HB = 256                  # samples per chunk (2 tiles)
# deltas whose product-multiplies run on GPSIMD/Pool (largest n first);
# the rest, and the whole reduce tree, run on DVE.
GP_DELTAS = ()
# deltas batched through the shared full-width scratch (small n: fewer,
# larger collapse ops). Must be a contiguous tail range.
BATCH_DELTAS = tuple(range(14, NE))
# which of the global tree levels (8, 4, 2) run on Pool
GP_TREE = ()


def _split_multiwaits(nc):
    """The walrus build accepts at most ONE sync wait per instruction.
    Hoist extra waits onto single-wait NoOps inserted immediately before the
    carrying instruction on the same engine."""
    n_extra = 0
    for fn in nc.m.functions:
        for blk in fn.blocks:
            insts = blk.instructions
            out = []
            for inst in insts:
                si = inst.sync_info
                waits = list(si.on_wait) if si is not None else []
                if len(waits) > 1:
                    for k, w in enumerate(waits[:-1]):
                        nop = bass_rust.InstNoOp(name=f"{inst.name}-sw{k}")
                        nop.engine = inst.engine
                        nop.bass_nofuse = True
                        nop.sync_info = bass_rust.SyncInfo(
                            on_wait=[w], on_update=[])
                        nc.register_instruction(nop, overwrite=True)
                        out.append(nop)
                        n_extra += 1
                    inst.sync_info = bass_rust.SyncInfo(
                        on_wait=[waits[-1]], on_update=list(si.on_update))
                out.append(inst)
            blk.instructions = out
    return n_extra


def _pair_maps():
    """Map my interaction row order (by distance delta, then i) to the
    reference np.triu row-major order, as an index array my_of_ref."""
    ref = {}
    k = 0
    for i in range(NE):
        for j in range(i + 1, NE):
            ref[(i, j)] = k
            k += 1
    mine = []
    for delta in range(1, NE):
        for i in range(NE - delta):
            mine.append(ref[(i, i + delta)])
    return np.array(mine, dtype=np.int64)  # mine[m] = ref index of my row m


# offsets of each delta's pair block inside the 351-row interaction layout
_OFFS = np.concatenate([[0], np.cumsum([NE - d for d in range(1, NE)])])


def build_nc(b_core=BC, hb=HB, loop_n=1, ablate=()):
    assert b_core % hb == 0 and hb % P == 0
    n_chunk = b_core // hb
    tpc = hb // P             # tiles per chunk
    n_tiles_all = b_core // P

    nc = bass.Bass()
    # ---- DRAM I/O ----
    xT = nc.dram_tensor("xT", [CD, b_core], F16, kind="ExternalInput")
    idx = nc.dram_tensor("idx", [b_core, NF], I32, kind="ExternalInput")
    tabs = nc.dram_tensor("tabs", [NF * V, D], F16, kind="ExternalInput")
    w1 = nc.dram_tensor("w1", [CD, 256], F16, kind="ExternalInput")
    b1 = nc.dram_tensor("b1", [P, 2], F32, kind="ExternalInput")
    w2 = nc.dram_tensor("w2", [256, P], F16, kind="ExternalInput")
    b2 = nc.dram_tensor("b2", [P, 1], F32, kind="ExternalInput")
    w3 = nc.dram_tensor("w3", [P, D], F16, kind="ExternalInput")
    b3 = nc.dram_tensor("b3", [D, 1], F32, kind="ExternalInput")
    w4 = nc.dram_tensor("w4", [TOPK, 512], F16, kind="ExternalInput")
    b4 = nc.dram_tensor("b4", [P, 4], F32, kind="ExternalInput")
    w5 = nc.dram_tensor("w5", [512, 256], F16, kind="ExternalInput")
    b5 = nc.dram_tensor("b5", [P, 2], F32, kind="ExternalInput")
    w6 = nc.dram_tensor("w6", [256, P], F16, kind="ExternalInput")
    b6 = nc.dram_tensor("b6", [P, 1], F32, kind="ExternalInput")
    w7 = nc.dram_tensor("w7", [P, 1], F16, kind="ExternalInput")
    b7 = nc.dram_tensor("b7", [1, 1], F32, kind="ExternalInput")
    scT = nc.dram_tensor("scT", [1, b_core], F32, kind="ExternalOutput")

    with tile.TileContext(nc) as tc:
        with (
            tc.tile_pool(name="const", bufs=1) as constp,
            tc.tile_pool(name="bot", bufs=1) as botp,
            tc.tile_pool(name="g", bufs=3) as gp,
            tc.tile_pool(name="prdg", bufs=2) as prdgp,
            tc.tile_pool(name="prdv", bufs=2) as prdvp,
            tc.tile_pool(name="ph", bufs=1) as php,
            tc.tile_pool(name="ph2", bufs=2) as ph2p,
            tc.tile_pool(name="trees", bufs=1) as treep,
            tc.tile_pool(name="intp", bufs=2) as intp,
            tc.tile_pool(name="fm", bufs=2) as fmp,
            tc.tile_pool(name="acts", bufs=1) as actp,
            tc.tile_pool(name="outp", bufs=2) as outp,
            tc.tile_pool(name="mmps", bufs=4, space="PSUM") as mmps,
            tc.tile_pool(name="tps", bufs=2, space="PSUM") as tps,
        ):
            from concourse.masks import make_identity
            ident = constp.tile([P, P], F16)
            make_identity(nc, ident[:])

            # ---- gather indices in one early DMA ----
            idxall = constp.tile([P, n_tiles_all, NF], I32)
            nc.sync.dma_start(
                idxall[:], idx[:].rearrange("(t p) f -> p t f", p=P))

            # ---- persistent weights in SBUF ----
            w1s = constp.tile([CD, 256], F16)
            nc.sync.dma_start(w1s[:], w1[:])
            b1s = constp.tile([P, 2], F32)
            nc.sync.dma_start(b1s[:], b1[:])
            w2s = constp.tile([P, 2, P], F16)
            nc.sync.dma_start(w2s[:], w2[:].rearrange("(c p) m -> p c m", p=P))
            b2s = constp.tile([P, 1], F32)
            nc.sync.dma_start(b2s[:], b2[:])
            w3s = constp.tile([P, D], F16)
            nc.sync.dma_start(w3s[:], w3[:])
            b3s = constp.tile([D, 1], F32)
            nc.sync.dma_start(b3s[:], b3[:])
            w4s = constp.tile([P, K_TILES, 512], F16)
            nc.sync.dma_start(w4s[:], w4[:].rearrange("(c p) m -> p c m", p=P))
            b4s = constp.tile([P, 4], F32)
            nc.sync.dma_start(b4s[:], b4[:])
            w5s = constp.tile([P, 4, 256], F16)
            nc.sync.dma_start(w5s[:], w5[:].rearrange("(c p) m -> p c m", p=P))
            b5s = constp.tile([P, 2], F32)
            nc.sync.dma_start(b5s[:], b5[:])
            w6s = constp.tile([P, 2, P], F16)
            nc.sync.dma_start(w6s[:], w6[:].rearrange("(c p) m -> p c m", p=P))
            b6s = constp.tile([P, 1], F32)
            nc.sync.dma_start(b6s[:], b6[:])
            w7s = constp.tile([P, 1], F16)
            nc.sync.dma_start(w7s[:], w7[:])
            b7s = constp.tile([1, 1], F32)
            nc.sync.dma_start(b7s[:], b7[:])

            def emit_body(lp=None):
                # ---------- bottom MLP for the whole core batch ----------
                BW = min(512, b_core)
                n_bot = b_core // BW
                bTs = []
                for nck in range(n_bot):
                    nsl = slice(nck * BW, (nck + 1) * BW)
                    xTs = botp.tile([CD, BW], F16, tag=f"xTs{nck % 2}")
                    nc.sync.dma_start(xTs[:], xT[:, nsl])
                    h1T = botp.tile([P, 2, BW], F16, tag=f"h1T{nck % 2}")
                    h2T = botp.tile([P, BW], F16, tag=f"h2T{nck % 2}")
                    bT = botp.tile([D, BW], F16, tag=f"bT{nck}")
                    bTs.append(bT)
                    for mc in range(2):
                        ps = mmps.tile([P, BW], F32, tag="mmps")
                        nc.tensor.matmul(
                            ps[:], w1s[:, mc * P:(mc + 1) * P], xTs[:],
                            start=True, stop=True)
                        nc.scalar.activation(
                            h1T[:, mc, :], ps[:],
                            mybir.ActivationFunctionType.Relu,
                            bias=b1s[:, mc:mc + 1])
                    ps = mmps.tile([P, BW], F32, tag="mmps")
                    for kc in range(2):
                        nc.tensor.matmul(
                            ps[:], w2s[:, kc, :], h1T[:, kc, :],
                            start=(kc == 0), stop=(kc == 1))
                    nc.scalar.activation(
                        h2T[:], ps[:],
                        mybir.ActivationFunctionType.Relu, bias=b2s[:, 0:1])
                    ps = mmps.tile([P, BW], F32, tag="mmps")
                    nc.tensor.matmul(
                        ps[:D], w3s[:], h2T[:], start=True, stop=True)
                    nc.scalar.activation(
                        bT[:], ps[:D],
                        mybir.ActivationFunctionType.Identity,
                        bias=b3s[:])

                # ---------- gather issue helper (per chunk) ----------
                def issue_gathers(h):
                    allg = gp.tile([P, tpc, NEP, D], F16, tag="allg")
                    for t in range(tpc):
                        tg = h * tpc + t
                        if 'gather' not in ablate:
                            # one indirect DMA per feature column (the HW
                            # SWDGE ucode only handles [128,1] offset APs)
                            for f in range(NF):
                                nc.gpsimd.indirect_dma_start(
                                    out=allg[:, t, f + 1, :],
                                    out_offset=None,
                                    in_=tabs[:],
                                    in_offset=bass.IndirectOffsetOnAxis(
                                        ap=idxall[:, tg, f:f + 1], axis=0))
                        if 'gather' in ablate:
                            nc.vector.memset(allg[:, t, 1:NE, :], 0.0)
                        # bottom output into slot 0 (DMA transpose)
                        bt_i, bt_o = (tg * P) // BW, (tg * P) % BW
                        nc.sync.dma_start_transpose(
                            allg[:, t, 0, :], bTs[bt_i][:, bt_o:bt_o + P])
                    # zero the pad feature slot 27
                    nc.vector.memset(allg[:, :, NE, :], 0.0)
                    return allg

                allgs = {0: issue_gathers(0)}
                if n_chunk > 1:
                    allgs[1] = issue_gathers(1)

                for h in range(n_chunk):
                    if h + 2 < n_chunk:
                        allgs[h + 2] = issue_gathers(h + 2)
                    allg = allgs.pop(h)
                    fm = fmp.tile([P, K_TILES, hb], F16, tag="fm")
                    inter = intp.tile([P, tpc, INTER_ROWS], F16, tag="inter")

                    # ---------- interactions (both tiles at once) ----------
                    prodH = ph2p.tile([P, tpc, NPP, 16], F16, tag="prodH")
                    nc.vector.memset(prodH[:, :, NPAIR, :], 0.0)
                    nb0 = int(_OFFS[BATCH_DELTAS[0] - 1])
                    nbat = NPAIR - nb0   # pairs in the batched tail
                    with nc.allow_low_precision(
                            reason="fp16 interactions; tolerance 2e-2"):
                        if 'inter' not in ablate:
                            # per-delta 64 -> 32 -> 16 for large/mid deltas
                            for delta in range(1, BATCH_DELTAS[0]):
                                n = NE - delta
                                off = int(_OFFS[delta - 1])
                                eng = (nc.gpsimd if delta in GP_DELTAS
                                       else nc.vector)
                                pp = prdgp if delta in GP_DELTAS else prdvp
                                nmax = NF
                                prd = pp.tile([P, tpc, nmax, D], F16,
                                              tag="prd")
                                eng.tensor_tensor(
                                    prd[:, :, :n, :], allg[:, :, 0:n, :],
                                    allg[:, :, delta:delta + n, :],
                                    op=mybir.AluOpType.mult)
                                p32 = pp.tile([P, tpc, nmax, 32], F16,
                                                tag="p32")
                                eng.tensor_tensor(
                                    p32[:, :, :n, :],
                                    prd[:, :, :n, 0:32],
                                    prd[:, :, :n, 32:64],
                                    op=mybir.AluOpType.add)
                                eng.tensor_tensor(
                                    prodH[:, :, off:off + n, :],
                                    p32[:, :, :n, 0:16],
                                    p32[:, :, :n, 16:32],
                                    op=mybir.AluOpType.add)
                            # batched tail: full-width mults, 2 collapse adds
                            pb = php.tile([P, tpc, nbat, D], F16, tag="pb")
                            for delta in BATCH_DELTAS:
                                n = NE - delta
                                off = int(_OFFS[delta - 1]) - nb0
                                nc.gpsimd.tensor_tensor(
                                    pb[:, :, off:off + n, :],
                                    allg[:, :, 0:n, :],
                                    allg[:, :, delta:delta + n, :],
                                    op=mybir.AluOpType.mult)
                            pb32 = php.tile([P, tpc, nbat, 32], F16,
                                            tag="pb32")
                            nc.vector.tensor_tensor(
                                pb32[:], pb[:, :, :, 0:32],
                                pb[:, :, :, 32:64], op=mybir.AluOpType.add)
                            nc.vector.tensor_tensor(
                                prodH[:, :, nb0:NPAIR, :],
                                pb32[:, :, :, 0:16], pb32[:, :, :, 16:32],
                                op=mybir.AluOpType.add)
                        else:
                            nc.vector.memset(prodH[:], 0.0)
                        # ---------- global tree 16 -> 1 ----------
                        t8 = treep.tile([P, tpc, NPP, 8], F16, tag="t8")
                        eng8 = nc.gpsimd if 8 in GP_TREE else nc.vector
                        eng8.tensor_tensor(
                            t8[:], prodH[:, :, :, 0:8], prodH[:, :, :, 8:16],
                            op=mybir.AluOpType.add)
                        t4 = treep.tile([P, tpc, NPP, 4], F16, tag="t4")
                        eng4 = nc.gpsimd if 4 in GP_TREE else nc.vector
                        eng4.tensor_tensor(
                            t4[:], t8[:, :, :, 0:4], t8[:, :, :, 4:8],
                            op=mybir.AluOpType.add)
                        t2 = treep.tile([P, tpc, NPP, 2], F16, tag="t2")
                        eng2 = nc.gpsimd if 2 in GP_TREE else nc.vector
                        eng2.tensor_tensor(
                            t2[:], t4[:, :, :, 0:2], t4[:, :, :, 2:4],
                            op=mybir.AluOpType.add)
                        nc.vector.tensor_tensor(
                            inter[:, :, 0:NPP].rearrange(
                                "p t (n o) -> p t n o", o=1),
                            t2[:, :, :, 0:1], t2[:, :, :, 1:2],
                            op=mybir.AluOpType.add)
                    # zero inter rows 352..383 (pad to 3 k-tiles)
                    nc.scalar.memzero(inter[:, :, NPP:INTER_ROWS])

                    # ---------- feature-major via PE transposes ----------
                    if 'tr' in ablate:
                        nc.scalar.memzero(fm[:])
                    if 'tr' not in ablate:
                        for t in range(tpc):
                            col = slice(t * P, (t + 1) * P)
                            for c2 in range(NEP // 4):
                                tp = tps.tile([P, 2, P], F16, tag="tp")
                                for u in range(2):
                                    c = 2 * c2 + u
                                    nc.tensor.transpose(
                                        tp[:, u, :],
                                        allg[:, t, 2 * c:2 * c + 2, :],
                                        ident[:])
                                nc.scalar.copy(
                                    fm[:, 2 * c2:2 * c2 + 2, col], tp[:])
                            tpi = tps.tile([P, 3, P], F16, tag="tpi")
                            for j in range(INTER_ROWS // P):
                                nc.tensor.transpose(
                                    tpi[:, j, :],
                                    inter[:, t, j * P:(j + 1) * P],
                                    ident[:])
                            nc.scalar.copy(
                                fm[:, NEP // 2:NEP // 2 + 3, col], tpi[:])

                    # ---------- top MLP (feature-major, fp16) ----------
                    t1T = actp.tile([P, 4, hb], F16, tag="t1T")
                    t2T = actp.tile([P, 2, hb], F16, tag="t2T")
                    t3T = actp.tile([P, hb], F16, tag="t3T")
                    for mc in range(4):
                        ps = mmps.tile([P, hb], F32, tag="mmps")
                        for kc in range(K_TILES):
                            nc.tensor.matmul(
                                ps[:], w4s[:, kc, mc * P:(mc + 1) * P],
                                fm[:, kc, :],
                                start=(kc == 0), stop=(kc == K_TILES - 1))
                        nc.scalar.activation(
                            t1T[:, mc, :], ps[:],
                            mybir.ActivationFunctionType.Relu,
                            bias=b4s[:, mc:mc + 1])
                    for mc in range(2):
                        ps = mmps.tile([P, hb], F32, tag="mmps")
                        for kc in range(4):
                            nc.tensor.matmul(
                                ps[:], w5s[:, kc, mc * P:(mc + 1) * P],
                                t1T[:, kc, :],
                                start=(kc == 0), stop=(kc == 3))
                        nc.scalar.activation(
                            t2T[:, mc, :], ps[:],
                            mybir.ActivationFunctionType.Relu,
                            bias=b5s[:, mc:mc + 1])
                    ps = mmps.tile([P, hb], F32, tag="mmps")
                    for kc in range(2):
                        nc.tensor.matmul(
                            ps[:], w6s[:, kc, :], t2T[:, kc, :],
                            start=(kc == 0), stop=(kc == 1))
                    nc.scalar.activation(
                        t3T[:], ps[:],
                        mybir.ActivationFunctionType.Relu, bias=b6s[:, 0:1])
                    ps7 = mmps.tile([P, hb], F32, tag="mmps")
                    nc.tensor.matmul(
                        ps7[:1], w7s[:], t3T[:], start=True, stop=True)
                    so = outp.tile([1, hb], F32, tag="so")
                    nc.vector.tensor_add(
                        so[:], ps7[:1], b7s[:].to_broadcast([1, hb]))
                    nc.sync.dma_start(scT[:, h * hb:(h + 1) * hb], so[:])

            for _rep in range(loop_n):
                emit_body()

    _split_multiwaits(nc)
    return nc


def prep_host(inputs, b_core=BC):
    """Fold BN, reorder W4, cast to fp16, build per-core input maps."""
    f = lambda a: np.asarray(a, dtype=np.float32)
    h = lambda a: np.ascontiguousarray(a, dtype=np.float16)
    continuous = f(inputs["continuous"])
    cat_idx = np.asarray(inputs["cat_idx"])
    tabs = h(np.asarray(inputs["emb_tables"]).reshape(NF * V, D))

    s1 = f(inputs["g1"]) * BN_INV
    w1f = h(f(inputs["W1"]) * s1[None, :])
    b1f = (f(inputs["b1"]) * s1 + f(inputs["be1"])).reshape(2, P).T.copy()
    s2 = f(inputs["g2"]) * BN_INV
    w2f = h(f(inputs["W2"]) * s2[None, :])
    b2f = (f(inputs["b2"]) * s2 + f(inputs["be2"])).reshape(1, P).T.copy()
    w3f = h(inputs["W3"])
    b3f = f(inputs["b3"]).reshape(D, 1)

    s4 = f(inputs["g4"]) * BN_INV
    W4 = f(inputs["W4"]) * s4[None, :]
    b4f = (f(inputs["b4"]) * s4 + f(inputs["be4"])).reshape(4, P).T.copy()
    my_of_ref = _pair_maps()
    W4m = np.zeros((TOPK, 512), dtype=np.float32)
    W4m[:NE * D] = W4[NPAIR:NPAIR + NE * D]            # flat part (27 feats)
    W4m[FLAT_ROWS + np.arange(NPAIR)] = W4[my_of_ref]  # interactions
    W4m = h(W4m)
    s5 = f(inputs["g5"]) * BN_INV
    w5f = h(f(inputs["W5"]) * s5[None, :])
    b5f = (f(inputs["b5"]) * s5 + f(inputs["be5"])).reshape(2, P).T.copy()
    s6 = f(inputs["g6"]) * BN_INV
    w6f = h(f(inputs["W6"]) * s6[None, :])
    b6f = (f(inputs["b6"]) * s6 + f(inputs["be6"])).reshape(1, P).T.copy()
    w7f = h(inputs["W7"])
    b7f = f(inputs["b7"]).reshape(1, 1)

    foffs = (np.arange(NF, dtype=np.int64) * V).astype(np.int32)
    in_maps = []
    ncores = B // b_core
    for c in range(ncores):
        sl = slice(c * b_core, (c + 1) * b_core)
        in_maps.append(dict(
            xT=h(continuous[sl].T),
            idx=np.ascontiguousarray(cat_idx[sl].astype(np.int32)
                                     + foffs[None, :]),
            tabs=tabs,
            w1=w1f, b1=b1f, w2=w2f, b2=b2f, w3=w3f, b3=b3f,
            w4=W4m, b4=b4f, w5=w5f, b5=b5f, w6=w6f, b6=b6f,
            w7=w7f, b7=b7f,
        ))
    return in_maps


_NC_CACHE = {}


def kernel(**inputs) -> np.ndarray:
    from concourse.bass_utils import run_bass_kernel_spmd

    key = (BC, HB)
    if key not in _NC_CACHE:
        _NC_CACHE[key] = build_nc(*key)
    nc = _NC_CACHE[key]
    in_maps = prep_host(inputs, BC)
    res = run_bass_kernel_spmd(nc, in_maps, core_ids=list(range(NCORES)))
    out = np.concatenate(
        [r["scT"].reshape(BC, 1) for r in res.results], axis=0)
    return out.astype(np.float32)
